# revision 53
# baseline (speedup 1.0000x reference)
"""Causal self-attention for B=4, L=2048, D=768, H=6 on 8 TRN2 NeuronCores.

Sharding: 8 cores = 4 batches x 2 head-groups (3 heads / 384 hidden each).
Each core computes, for its (batch, head-group):
  QT/KT = (x @ W{q,k})^T + b            [128d x L per head, fp32r]
  V     = x @ Wv                        [L x 384, fp32r]
  per head, per 512-wide q-group:
    S^T  = K_blk @ Q^T                  (PE, fp32r, causal block-skip)
    A^T  = exp(S^T / sqrt(128))         (ACT, masked on diagonal blocks)
    O^T += V_blk^T @ A^T                (PE)  + row-sums via ones-matmul
    O^T *= 1/sums  (sums broadcast over partitions via rank-1 matmul)
  Y_part = O @ Wo_slice                 [L x 768 partial]
Host sums the two head-group partials per batch and adds the bias terms
(bv @ Wo + bo); bq/bk are applied on-device (zero-cost per-partition add).

All matmuls run in float32r (full PE rate, ~1e-4 rel err); softmax math in
fp32. exp needs no max-subtraction: scores/sqrt(128) stay in [-10, 10] for
normally-distributed inputs, well inside fp32 exp range.
"""

import math

import numpy as np

import concourse.bacc as bacc
import concourse.mybir as mybir
import concourse.tile as tile
from concourse.bass_utils import run_bass_kernel_spmd

F32 = mybir.dt.float32
F32R = mybir.dt.float32r
EXP = mybir.ActivationFunctionType.Exp

B = 4
L = 2048
D = 768
HEADS = 6
HD = 128
HPC = 3          # heads per core
HG = HPC * HD    # 384: per-core slice of the hidden dim
CB = D // 128    # 6 contraction chunks
SCALE = 1.0 / math.sqrt(HD)
N_CORES = 8


def build_nc(L_=L):
    """Build + compile the per-core Bass program (same program on all cores)."""
    LBn = L_ // 128   # 128-row L blocks
    NQG = L_ // 512   # 512-wide q groups

    nc = bacc.Bacc("TRN2", target_bir_lowering=False, debug=False)
    x_d = nc.dram_tensor("x", [L_, D], F32, kind="ExternalInput").ap()
    wq_d = nc.dram_tensor("wq", [D, HG], F32, kind="ExternalInput").ap()
    wk_d = nc.dram_tensor("wk", [D, HG], F32, kind="ExternalInput").ap()
    wv_d = nc.dram_tensor("wv", [D, HG], F32, kind="ExternalInput").ap()
    wo_d = nc.dram_tensor("wo", [HG, D], F32, kind="ExternalInput").ap()
    bq_d = nc.dram_tensor("bq", [HG], F32, kind="ExternalInput").ap()
    bk_d = nc.dram_tensor("bk", [HG], F32, kind="ExternalInput").ap()
    ident_d = nc.dram_tensor("ident", [128, 128], F32, kind="ExternalInput").ap()
    maskf_d = nc.dram_tensor("maskf", [128, 896], F32, kind="ExternalInput").ap()
    y_d = nc.dram_tensor("y", [L_, D], F32, kind="ExternalOutput").ap()

    with tile.TileContext(nc) as tc:
        with (
            tc.tile_pool(name="persist", bufs=1) as pp,
            tc.tile_pool(name="qkv_sb", bufs=1) as pqkv,
        ):
            # constants go on the SWDGE (gpsimd) queue so the HWDGE queue's
            # first descriptors are the x chunks the PE transposes wait on
            ident = pp.tile([128, 128], F32R)
            nc.sync.dma_start(ident, ident_d.bitcast(F32R))
            maskf = pp.tile([128, 896], F32R)
            bq_sb = pp.tile([128, HPC], F32)
            bk_sb = pp.tile([128, HPC], F32)
            nc.gpsimd.dma_start(bq_sb, bq_d.rearrange("(h p) -> p h", p=128))
            nc.gpsimd.dma_start(bk_sb, bk_d.rearrange("(h p) -> p h", p=128))
            # dummy exp: pulls the ACT Exp-table load off the QKV->attention
            # transition and into the startup DMA shadow
            warm = pp.tile([1, 1], F32)
            nc.scalar.activation(warm, ident[:1, :1], EXP)

            q_t = pqkv.tile([128, HPC, L_], F32R)   # Q^T: [d, (head, L)]
            k_t = pqkv.tile([128, HPC, L_], F32R)   # K^T
            v_t = pqkv.tile([128, LBn, HG], F32R)   # V:  [k-in-block, (block, hd)]
            o_t = pqkv.tile([128, HPC, L_], F32R)   # O^T (normalized)

            # ---- phase 1: load x, transpose to x^T, QKV projections ----
            with (
                tc.tile_pool(name="w_sb", bufs=1) as pw,
                tc.tile_pool(name="x_nat", bufs=8) as px,
                tc.tile_pool(name="xT", bufs=2) as pxt,
                tc.tile_pool(name="ps_t", bufs=2, space="PSUM") as ps_t,
                tc.tile_pool(name="ps_qk", bufs=2, space="PSUM") as ps_qk,
                tc.tile_pool(name="ps_v", bufs=2, space="PSUM") as ps_v,
            ):
                # per-128-row x tiles: fine-grained DMA→transpose pipelining
                def load_xb(g, b):
                    # alternate between the two HWDGE queues (SP / Activation)
                    # to parallelize descriptor generation and transfers
                    xb = px.tile([128, D], F32R, tag="xn")
                    r0 = g * 512 + b * 128
                    eng = nc.scalar if (g == 0 and b % 2 == 1) else nc.sync
                    eng.dma_start(
                        xb,
                        x_d.bitcast(F32R)[r0 : r0 + 128, :].rearrange(
                            "(o p) c -> p o c", p=128
                        )[:, 0],
                    )
                    return xb

                xbs = [load_xb(0, b) for b in range(4)]
                wq_sb = pw.tile([128, CB, HG], F32R)
                wk_sb = pw.tile([128, CB, HG], F32R)
                wv_sb = pw.tile([128, CB, HG], F32R)
                for w_sb, w_d in ((wq_sb, wq_d), (wk_sb, wk_d), (wv_sb, wv_d)):
                    nc.sync.dma_start(
                        w_sb, w_d.bitcast(F32R).rearrange("(c p) d -> p c d", p=128)
                    )

                def emit_transposes(g, xn):
                    # b-major groups: each PSUM group + copy depends on a
                    # single x row-block DMA, so the PE transposes stream in
                    # lockstep with the arriving sub-DMAs
                    xt = pxt.tile([128, CB, 512], F32R, name="xt")
                    for b in range(4):
                        for c0, cw in ((0, 4), (4, 2)):
                            pt = ps_t.tile([128, cw, 128], F32R, name="pt")
                            for ci in range(cw):
                                c = c0 + ci
                                nc.tensor.transpose(
                                    pt[:, ci, :],
                                    xn[b][:, c * 128 : (c + 1) * 128],
                                    ident,
                                )
                            nc.vector.tensor_copy(
                                xt[:, c0 : c0 + cw, b * 128 : (b + 1) * 128], pt
                            )
                    return xt

                xt = emit_transposes(0, xbs)
                for g in range(NQG):
                    if g + 1 < NQG:  # prefetch next chunk
                        xbs = [load_xb(g + 1, b) for b in range(4)]
                    qsl = slice(g * 512, (g + 1) * 512)
                    for h in range(HPC):
                        hsl = slice(h * 128, (h + 1) * 128)
                        pq = ps_qk.tile([128, 512], F32, tag="pq")
                        for c in range(CB):
                            nc.tensor.matmul(
                                pq, wq_sb[:, c, hsl], xt[:, c, :],
                                start=(c == 0), stop=(c == CB - 1),
                            )
                        nc.scalar.activation(
                            q_t[:, h, qsl], pq,
                            mybir.ActivationFunctionType.Identity,
                            bias=bq_sb[:, h : h + 1],
                        )
                        pk = ps_qk.tile([128, 512], F32, tag="pk")
                        for c in range(CB):
                            nc.tensor.matmul(
                                pk, wk_sb[:, c, hsl], xt[:, c, :],
                                start=(c == 0), stop=(c == CB - 1),
                            )
                        nc.scalar.activation(
                            k_t[:, h, qsl], pk,
                            mybir.ActivationFunctionType.Identity,
                            bias=bk_sb[:, h : h + 1],
                        )
                    # transposes for the next chunk run on the PE here, so
                    # their PSUM->SBUF copies land while the V matmuls run
                    xt_next = emit_transposes(g + 1, xbs) if g + 1 < NQG else None
                    for b in range(4):
                        lb = g * 4 + b
                        pv = ps_v.tile([128, HG], F32)
                        for c in range(CB):
                            nc.tensor.matmul(
                                pv, xt[:, c, b * 128 : (b + 1) * 128], wv_sb[:, c, :],
                                start=(c == 0), stop=(c == CB - 1),
                            )
                        nc.vector.tensor_copy(v_t[:, lb, :], pv)
                    xt = xt_next

            # ---- phase 2: attention + output projection ----
            with (
                tc.tile_pool(name="attn_sb", bufs=1) as pa,
                tc.tile_pool(name="at_pool", bufs=8) as pat,
                tc.tile_pool(name="nrm_sb", bufs=3) as pn,
                tc.tile_pool(name="y_pool", bufs=3) as py_,
                tc.tile_pool(name="ps_s", bufs=2, space="PSUM") as ps_s,
                tc.tile_pool(name="ps_o", bufs=2, space="PSUM") as ps_o,
                tc.tile_pool(name="ps_n", bufs=1, space="PSUM") as ps_n,
                tc.tile_pool(name="ps_y", bufs=1, space="PSUM") as ps_y,
            ):
                # maskf[p, c] = 1.0 if c >= p + 384 else 0.0; diagonal-block
                # mask for block i (0..3) is maskf[:, 384-128i : 896-128i].
                # maskf[:, 768:896] is all-ones: also used as the stationary
                # of the broadcast row-sum matmuls. Loaded here, off the
                # startup critical path.
                nc.sync.dma_start(maskf, maskf_d.bitcast(F32R))
                wo_sb = pa.tile([128, HPC, D], F32R)
                nc.sync.dma_start(
                    wo_sb, wo_d.bitcast(F32R).rearrange("(h p) e -> p h e", p=128)
                )
                # Flat software-pipelined stream over all (g, h, j) batches.
                # Per batch: S-matmuls -> exp (ACT) -> mask (DVE, diag only)
                # -> PV + row-sum matmuls. The S-matmuls of batch m+1 are
                # emitted before the PV of batch m, so the PE queue always
                # has an exp-independent batch in front of it, and the
                # finalize / projection work (which trails DVE results) is
                # emitted a batch or two late to avoid head-of-line blocks.
                flat = []
                for g in range(NQG):
                    nb = 2 * (g + 1)
                    order = list(range(nb))
                    for h in range(HPC):
                        for pos, j in enumerate(order):
                            flat.append((g, h, j, pos == nb - 1, pos == 0))
                state = {}
                pending = []  # (delay, closure)

                def emit_S(m):
                    g, h, j, last, first = flat[m]
                    ps = ps_s.tile([128, 2, 512], F32, tag="ps")
                    for t in range(2):
                        kb = 2 * j + t
                        nc.tensor.matmul(
                            ps[:, t, :],
                            k_t[:, h, kb * 128 : (kb + 1) * 128],
                            q_t[:, h, g * 512 : (g + 1) * 512],
                            start=True, stop=True,
                        )
                    state[m] = ps

                def emit_rest(m):
                    g, h, j, last, first = flat[m]
                    ps = state.pop(m)
                    if first:
                        state[("po", g, h)] = ps_o.tile([128, 512], F32, tag="po", name="po")
                        state[("sm", g, h)] = ps_n.tile([128, 512], F32, tag="nrm", name="psums")
                    po = state[("po", g, h)]
                    psums = state[("sm", g, h)]
                    at = pat.tile([128, 2, 512], F32R)
                    if last:
                        # split: halves the exp latency gating this group's
                        # finalize chain
                        nc.scalar.activation(at[:, 0, :], ps[:, 0, :], EXP, scale=SCALE)
                        nc.scalar.activation(at[:, 1, :], ps[:, 1, :], EXP, scale=SCALE)
                    else:
                        nc.scalar.activation(at, ps, EXP, scale=SCALE)
                    for t in range(2):
                        kb = 2 * j + t
                        i = kb - 4 * g
                        if i >= 0:  # block overlapping the causal diagonal
                            # only cols < 128(i+1) need masking: zero region
                            # [0,128i) + triangle [128i,128i+128)
                            off = 384 - 128 * i
                            w = 128 * (i + 1)
                            nc.vector.tensor_mul(
                                at[:, t, :w], at[:, t, :w], maskf[:, off : off + w]
                            )
                        # diag blocks i=1,2: cols < 128i are zero, skip them
                        # (i=3 stays full: N=128 loses the fp32r fast path;
                        # the first-emitted matmul stays full: PSUM init)
                        st, sp = first and t == 0, last and t == 1
                        c0 = 128 * i if i in (1, 2) and not st else 0
                        nc.tensor.matmul(
                            po[:, c0:],
                            v_t[:, kb, h * 128 : (h + 1) * 128],
                            at[:, t, c0:],
                            start=st, stop=sp,
                        )
                        nc.tensor.matmul(
                            psums[:, c0:],
                            maskf[:, 768:896],
                            at[:, t, c0:],
                            start=st, stop=sp,
                        )

                def emit_finalize(g, h):
                    def run():
                        po = state.pop(("po", g, h))
                        psums = state.pop(("sm", g, h))
                        recip = pn.tile([128, 512], F32, tag="recip")
                        nc.vector.reciprocal(recip, psums)
                        nc.vector.tensor_mul(
                            o_t[:, h, g * 512 : (g + 1) * 512], po, recip
                        )
                    return run

                def emit_proj(g):
                    def run():
                        # the last group's projection runs exposed after all
                        # attention work; borrow the then-idle ps_s slots to
                        # triple-buffer it
                        final = g == NQG - 1
                        for b in range(4):
                            lb = g * 4 + b
                            lsl = slice(lb * 128, (lb + 1) * 128)
                            ysb = py_.tile([128, D], F32, tag="ysb")
                            for eh in range(2):
                                pool = ps_s if final and (b + eh) % 2 else ps_y
                                pyp = pool.tile(
                                    [128, 384], F32,
                                    tag="ps" if pool is ps_s else "pyp",
                                    name="pyp",
                                )
                                for h in range(HPC):
                                    nc.tensor.matmul(
                                        pyp,
                                        o_t[:, h, lsl],
                                        wo_sb[:, h, eh * 384 : (eh + 1) * 384],
                                        start=(h == 0), stop=(h == HPC - 1),
                                    )
                                nc.vector.tensor_copy(
                                    ysb[:, eh * 384 : (eh + 1) * 384], pyp
                                )
                            nc.sync.dma_start(y_d[lb * 128 : (lb + 1) * 128, :], ysb)
                    return run

                emit_S(0)
                for m in range(len(flat)):
                    if m + 1 < len(flat):
                        emit_S(m + 1)
                    nxt = []
                    for d, fn in pending:
                        if d <= 0:
                            fn()
                        else:
                            nxt.append((d - 1, fn))
                    pending = nxt
                    emit_rest(m)
                    g, h, j, last, first = flat[m]
                    if last:
                        pending.append((1, emit_finalize(g, h)))
                        if h == HPC - 1:
                            pending.append((2, emit_proj(g)))
                for d, fn in sorted(pending, key=lambda p: p[0]):
                    fn()

    nc.compile()
    return nc


_NC_CACHE = {}


def _get_nc(L_=L):
    if L_ not in _NC_CACHE:
        _NC_CACHE[L_] = build_nc(L_)
    return _NC_CACHE[L_]


def run_sharded(inputs, L_=L, trace=False):
    """Shard inputs over 8 cores, run, return (list of per-core y, results obj)."""
    x = np.ascontiguousarray(inputs["x_input"], dtype=np.float32)
    ident = np.eye(128, dtype=np.float32)
    maskf = (np.arange(896)[None, :] >= np.arange(128)[:, None] + 384).astype(
        np.float32
    )
    in_maps = []
    for c in range(N_CORES):
        b, gslice = c // 2, slice((c % 2) * HG, (c % 2) * HG + HG)
        in_maps.append(
            {
                "x": x[b],
                "ident": ident,
                "maskf": maskf,
                "wq": np.ascontiguousarray(inputs["Wq"][:, gslice], np.float32),
                "wk": np.ascontiguousarray(inputs["Wk"][:, gslice], np.float32),
                "wv": np.ascontiguousarray(inputs["Wv"][:, gslice], np.float32),
                "wo": np.ascontiguousarray(inputs["Wo"][gslice, :], np.float32),
                "bq": np.ascontiguousarray(inputs["bq"][gslice], np.float32),
                "bk": np.ascontiguousarray(inputs["bk"][gslice], np.float32),
            }
        )
    nc = _get_nc(L_)
    try:
        res = run_bass_kernel_spmd(nc, in_maps, list(range(N_CORES)), trace=trace)
    except Exception:
        # transient device faults (NRT_EXEC_UNIT_UNRECOVERABLE etc.): one retry
        res = run_bass_kernel_spmd(nc, in_maps, list(range(N_CORES)), trace=trace)
    return res


def kernel(**inputs) -> np.ndarray:
    res = run_sharded(inputs)
    # host-side unshard: sum the two head-group partials per batch; add the
    # bias terms that commute out of the device computation exactly:
    # softmax rows sum to 1, so  A @ (xWv + bv) Wo + bo = A(xWv)Wo + bv@Wo + bo
    bias = (
        np.asarray(inputs["bv"], np.float32) @ np.asarray(inputs["Wo"], np.float32)
        + np.asarray(inputs["bo"], np.float32)
    )
    out = np.empty((B, L, D), dtype=np.float32)
    for b in range(B):
        out[b] = res.results[2 * b]["y"] + res.results[2 * b + 1]["y"] + bias
    return out


# revision 54
# speedup vs baseline: 1.0056x; 1.0056x over previous
"""Causal self-attention for B=4, L=2048, D=768, H=6 on 8 TRN2 NeuronCores.

Sharding: 8 cores = 4 batches x 2 head-groups (3 heads / 384 hidden each).
Each core computes, for its (batch, head-group):
  QT/KT = (x @ W{q,k})^T + b            [128d x L per head, fp32r]
  V     = x @ Wv                        [L x 384, fp32r]
  per head, per 512-wide q-group:
    S^T  = K_blk @ Q^T                  (PE, fp32r, causal block-skip)
    A^T  = exp(S^T / sqrt(128))         (ACT, masked on diagonal blocks)
    O^T += V_blk^T @ A^T                (PE)  + row-sums via ones-matmul
    O^T *= 1/sums  (sums broadcast over partitions via rank-1 matmul)
  Y_part = O @ Wo_slice                 [L x 768 partial]
Host sums the two head-group partials per batch and adds the bias terms
(bv @ Wo + bo); bq/bk are applied on-device (zero-cost per-partition add).

All matmuls run in float32r (full PE rate, ~1e-4 rel err); softmax math in
fp32. exp needs no max-subtraction: scores/sqrt(128) stay in [-10, 10] for
normally-distributed inputs, well inside fp32 exp range.
"""

import math

import numpy as np

import concourse.bacc as bacc
import concourse.mybir as mybir
import concourse.tile as tile
from concourse.bass_utils import run_bass_kernel_spmd

F32 = mybir.dt.float32
F32R = mybir.dt.float32r
EXP = mybir.ActivationFunctionType.Exp

B = 4
L = 2048
D = 768
HEADS = 6
HD = 128
HPC = 3          # heads per core
HG = HPC * HD    # 384: per-core slice of the hidden dim
CB = D // 128    # 6 contraction chunks
SCALE = 1.0 / math.sqrt(HD)
N_CORES = 8


def build_nc(L_=L):
    """Build + compile the per-core Bass program (same program on all cores)."""
    LBn = L_ // 128   # 128-row L blocks
    NQG = L_ // 512   # 512-wide q groups

    nc = bacc.Bacc("TRN2", target_bir_lowering=False, debug=False)
    x_d = nc.dram_tensor("x", [L_, D], F32, kind="ExternalInput").ap()
    wq_d = nc.dram_tensor("wq", [D, HG], F32, kind="ExternalInput").ap()
    wk_d = nc.dram_tensor("wk", [D, HG], F32, kind="ExternalInput").ap()
    wv_d = nc.dram_tensor("wv", [D, HG], F32, kind="ExternalInput").ap()
    wo_d = nc.dram_tensor("wo", [HG, D], F32, kind="ExternalInput").ap()
    bq_d = nc.dram_tensor("bq", [HG], F32, kind="ExternalInput").ap()
    bk_d = nc.dram_tensor("bk", [HG], F32, kind="ExternalInput").ap()
    ident_d = nc.dram_tensor("ident", [128, 128], F32, kind="ExternalInput").ap()
    maskf_d = nc.dram_tensor("maskf", [128, 896], F32, kind="ExternalInput").ap()
    y_d = nc.dram_tensor("y", [L_, D], F32, kind="ExternalOutput").ap()

    with tile.TileContext(nc) as tc:
        with (
            tc.tile_pool(name="persist", bufs=1) as pp,
            tc.tile_pool(name="qkv_sb", bufs=1) as pqkv,
        ):
            # constants go on the SWDGE (gpsimd) queue so the HWDGE queue's
            # first descriptors are the x chunks the PE transposes wait on
            ident = pp.tile([128, 128], F32R)
            nc.gpsimd.dma_start(ident, ident_d.bitcast(F32R))
            maskf = pp.tile([128, 896], F32R)
            bq_sb = pp.tile([128, HPC], F32)
            bk_sb = pp.tile([128, HPC], F32)
            nc.gpsimd.dma_start(bq_sb, bq_d.rearrange("(h p) -> p h", p=128))
            nc.gpsimd.dma_start(bk_sb, bk_d.rearrange("(h p) -> p h", p=128))
            # dummy exp: pulls the ACT Exp-table load off the QKV->attention
            # transition and into the startup DMA shadow
            warm = pp.tile([1, 1], F32)
            nc.scalar.activation(warm, ident[:1, :1], EXP)

            q_t = pqkv.tile([128, HPC, L_], F32R)   # Q^T: [d, (head, L)]
            k_t = pqkv.tile([128, HPC, L_], F32R)   # K^T
            v_t = pqkv.tile([128, LBn, HG], F32R)   # V:  [k-in-block, (block, hd)]
            o_t = pqkv.tile([128, HPC, L_], F32R)   # O^T (normalized)

            # ---- phase 1: load x, transpose to x^T, QKV projections ----
            with (
                tc.tile_pool(name="w_sb", bufs=1) as pw,
                tc.tile_pool(name="x_nat", bufs=8) as px,
                tc.tile_pool(name="xT", bufs=2) as pxt,
                tc.tile_pool(name="ps_t", bufs=2, space="PSUM") as ps_t,
                tc.tile_pool(name="ps_qk", bufs=2, space="PSUM") as ps_qk,
                tc.tile_pool(name="ps_v", bufs=2, space="PSUM") as ps_v,
            ):
                # per-128-row x tiles: fine-grained DMA→transpose pipelining
                def load_xb(g, b):
                    # alternate between the two HWDGE queues (SP / Activation)
                    # to parallelize descriptor generation and transfers
                    xb = px.tile([128, D], F32R, tag="xn")
                    r0 = g * 512 + b * 128
                    eng = nc.scalar if (g == 0 and b % 2 == 1) else nc.sync
                    eng.dma_start(
                        xb,
                        x_d.bitcast(F32R)[r0 : r0 + 128, :].rearrange(
                            "(o p) c -> p o c", p=128
                        )[:, 0],
                    )
                    return xb

                xbs = [load_xb(0, b) for b in range(4)]
                wq_sb = pw.tile([128, CB, HG], F32R)
                wk_sb = pw.tile([128, CB, HG], F32R)
                wv_sb = pw.tile([128, CB, HG], F32R)
                for w_sb, w_d in ((wq_sb, wq_d), (wk_sb, wk_d), (wv_sb, wv_d)):
                    nc.sync.dma_start(
                        w_sb, w_d.bitcast(F32R).rearrange("(c p) d -> p c d", p=128)
                    )

                def emit_transposes(g, xn):
                    # b-major groups: each PSUM group + copy depends on a
                    # single x row-block DMA, so the PE transposes stream in
                    # lockstep with the arriving sub-DMAs
                    xt = pxt.tile([128, CB, 512], F32R, name="xt")
                    for b in range(4):
                        for c0, cw in ((0, 4), (4, 2)):
                            pt = ps_t.tile([128, cw, 128], F32R, name="pt")
                            for ci in range(cw):
                                c = c0 + ci
                                nc.tensor.transpose(
                                    pt[:, ci, :],
                                    xn[b][:, c * 128 : (c + 1) * 128],
                                    ident,
                                )
                            nc.vector.tensor_copy(
                                xt[:, c0 : c0 + cw, b * 128 : (b + 1) * 128], pt
                            )
                    return xt

                xt = emit_transposes(0, xbs)
                for g in range(NQG):
                    if g + 1 < NQG:  # prefetch next chunk
                        xbs = [load_xb(g + 1, b) for b in range(4)]
                    qsl = slice(g * 512, (g + 1) * 512)
                    for h in range(HPC):
                        hsl = slice(h * 128, (h + 1) * 128)
                        pq = ps_qk.tile([128, 512], F32, tag="pq")
                        for c in range(CB):
                            nc.tensor.matmul(
                                pq, wq_sb[:, c, hsl], xt[:, c, :],
                                start=(c == 0), stop=(c == CB - 1),
                            )
                        nc.scalar.activation(
                            q_t[:, h, qsl], pq,
                            mybir.ActivationFunctionType.Identity,
                            bias=bq_sb[:, h : h + 1],
                        )
                        pk = ps_qk.tile([128, 512], F32, tag="pk")
                        for c in range(CB):
                            nc.tensor.matmul(
                                pk, wk_sb[:, c, hsl], xt[:, c, :],
                                start=(c == 0), stop=(c == CB - 1),
                            )
                        nc.scalar.activation(
                            k_t[:, h, qsl], pk,
                            mybir.ActivationFunctionType.Identity,
                            bias=bk_sb[:, h : h + 1],
                        )
                    # transposes for the next chunk run on the PE here, so
                    # their PSUM->SBUF copies land while the V matmuls run
                    xt_next = emit_transposes(g + 1, xbs) if g + 1 < NQG else None
                    for b in range(4):
                        lb = g * 4 + b
                        pv = ps_v.tile([128, HG], F32)
                        for c in range(CB):
                            nc.tensor.matmul(
                                pv, xt[:, c, b * 128 : (b + 1) * 128], wv_sb[:, c, :],
                                start=(c == 0), stop=(c == CB - 1),
                            )
                        nc.vector.tensor_copy(v_t[:, lb, :], pv)
                    xt = xt_next

            # ---- phase 2: attention + output projection ----
            with (
                tc.tile_pool(name="attn_sb", bufs=1) as pa,
                tc.tile_pool(name="at_pool", bufs=8) as pat,
                tc.tile_pool(name="nrm_sb", bufs=3) as pn,
                tc.tile_pool(name="y_pool", bufs=3) as py_,
                tc.tile_pool(name="ps_s", bufs=2, space="PSUM") as ps_s,
                tc.tile_pool(name="ps_o", bufs=2, space="PSUM") as ps_o,
                tc.tile_pool(name="ps_n", bufs=1, space="PSUM") as ps_n,
                tc.tile_pool(name="ps_y", bufs=1, space="PSUM") as ps_y,
            ):
                # maskf[p, c] = 1.0 if c >= p + 384 else 0.0; diagonal-block
                # mask for block i (0..3) is maskf[:, 384-128i : 896-128i].
                # maskf[:, 768:896] is all-ones: also used as the stationary
                # of the broadcast row-sum matmuls. Loaded here, off the
                # startup critical path.
                nc.sync.dma_start(maskf, maskf_d.bitcast(F32R))
                wo_sb = pa.tile([128, HPC, D], F32R)
                nc.sync.dma_start(
                    wo_sb, wo_d.bitcast(F32R).rearrange("(h p) e -> p h e", p=128)
                )
                # Flat software-pipelined stream over all (g, h, j) batches.
                # Per batch: S-matmuls -> exp (ACT) -> mask (DVE, diag only)
                # -> PV + row-sum matmuls. The S-matmuls of batch m+1 are
                # emitted before the PV of batch m, so the PE queue always
                # has an exp-independent batch in front of it, and the
                # finalize / projection work (which trails DVE results) is
                # emitted a batch or two late to avoid head-of-line blocks.
                flat = []
                for g in range(NQG):
                    nb = 2 * (g + 1)
                    order = list(range(nb))
                    for h in range(HPC):
                        for pos, j in enumerate(order):
                            flat.append((g, h, j, pos == nb - 1, pos == 0))
                state = {}
                pending = []  # (delay, closure)

                def emit_S(m):
                    g, h, j, last, first = flat[m]
                    ps = ps_s.tile([128, 2, 512], F32, tag="ps")
                    for t in range(2):
                        kb = 2 * j + t
                        nc.tensor.matmul(
                            ps[:, t, :],
                            k_t[:, h, kb * 128 : (kb + 1) * 128],
                            q_t[:, h, g * 512 : (g + 1) * 512],
                            start=True, stop=True,
                        )
                    state[m] = ps

                def emit_rest(m):
                    g, h, j, last, first = flat[m]
                    ps = state.pop(m)
                    if first:
                        state[("po", g, h)] = ps_o.tile([128, 512], F32, tag="po", name="po")
                        state[("sm", g, h)] = ps_n.tile([128, 512], F32, tag="nrm", name="psums")
                    po = state[("po", g, h)]
                    psums = state[("sm", g, h)]
                    at = pat.tile([128, 2, 512], F32R)
                    if last:
                        # split: halves the exp latency gating this group's
                        # finalize chain
                        nc.scalar.activation(at[:, 0, :], ps[:, 0, :], EXP, scale=SCALE)
                        nc.scalar.activation(at[:, 1, :], ps[:, 1, :], EXP, scale=SCALE)
                    else:
                        nc.scalar.activation(at, ps, EXP, scale=SCALE)
                    for t in range(2):
                        kb = 2 * j + t
                        i = kb - 4 * g
                        if i >= 0:  # block overlapping the causal diagonal
                            # only cols < 128(i+1) need masking: zero region
                            # [0,128i) + triangle [128i,128i+128)
                            off = 384 - 128 * i
                            w = 128 * (i + 1)
                            nc.vector.tensor_mul(
                                at[:, t, :w], at[:, t, :w], maskf[:, off : off + w]
                            )
                        # diag blocks i=1,2: cols < 128i are zero, skip them
                        # (i=3 stays full: N=128 loses the fp32r fast path;
                        # the first-emitted matmul stays full: PSUM init)
                        st, sp = first and t == 0, last and t == 1
                        c0 = 128 * i if i in (1, 2) and not st else 0
                        nc.tensor.matmul(
                            po[:, c0:],
                            v_t[:, kb, h * 128 : (h + 1) * 128],
                            at[:, t, c0:],
                            start=st, stop=sp,
                        )
                        nc.tensor.matmul(
                            psums[:, c0:],
                            maskf[:, 768:896],
                            at[:, t, c0:],
                            start=st, stop=sp,
                        )

                def emit_finalize(g, h):
                    def run():
                        po = state.pop(("po", g, h))
                        psums = state.pop(("sm", g, h))
                        recip = pn.tile([128, 512], F32, tag="recip")
                        nc.vector.reciprocal(recip, psums)
                        nc.vector.tensor_mul(
                            o_t[:, h, g * 512 : (g + 1) * 512], po, recip
                        )
                    return run

                def emit_proj(g):
                    def run():
                        # the last group's projection runs exposed after all
                        # attention work; borrow the then-idle ps_s slots to
                        # triple-buffer it
                        final = g == NQG - 1
                        for b in range(4):
                            lb = g * 4 + b
                            lsl = slice(lb * 128, (lb + 1) * 128)
                            ysb = py_.tile([128, D], F32, tag="ysb")
                            for eh in range(2):
                                pool = ps_s if final and (b + eh) % 2 else ps_y
                                pyp = pool.tile(
                                    [128, 384], F32,
                                    tag="ps" if pool is ps_s else "pyp",
                                    name="pyp",
                                )
                                for h in range(HPC):
                                    nc.tensor.matmul(
                                        pyp,
                                        o_t[:, h, lsl],
                                        wo_sb[:, h, eh * 384 : (eh + 1) * 384],
                                        start=(h == 0), stop=(h == HPC - 1),
                                    )
                                nc.vector.tensor_copy(
                                    ysb[:, eh * 384 : (eh + 1) * 384], pyp
                                )
                            nc.sync.dma_start(y_d[lb * 128 : (lb + 1) * 128, :], ysb)
                    return run

                emit_S(0)
                for m in range(len(flat)):
                    if m + 1 < len(flat):
                        emit_S(m + 1)
                    nxt = []
                    for d, fn in pending:
                        if d <= 0:
                            fn()
                        else:
                            nxt.append((d - 1, fn))
                    pending = nxt
                    emit_rest(m)
                    g, h, j, last, first = flat[m]
                    if last:
                        pending.append((1, emit_finalize(g, h)))
                        if h == HPC - 1:
                            pending.append((2, emit_proj(g)))
                for d, fn in sorted(pending, key=lambda p: p[0]):
                    fn()

    nc.compile()
    return nc


_NC_CACHE = {}


def _get_nc(L_=L):
    if L_ not in _NC_CACHE:
        _NC_CACHE[L_] = build_nc(L_)
    return _NC_CACHE[L_]


def run_sharded(inputs, L_=L, trace=False):
    """Shard inputs over 8 cores, run, return (list of per-core y, results obj)."""
    x = np.ascontiguousarray(inputs["x_input"], dtype=np.float32)
    ident = np.eye(128, dtype=np.float32)
    maskf = (np.arange(896)[None, :] >= np.arange(128)[:, None] + 384).astype(
        np.float32
    )
    in_maps = []
    for c in range(N_CORES):
        b, gslice = c // 2, slice((c % 2) * HG, (c % 2) * HG + HG)
        in_maps.append(
            {
                "x": x[b],
                "ident": ident,
                "maskf": maskf,
                "wq": np.ascontiguousarray(inputs["Wq"][:, gslice], np.float32),
                "wk": np.ascontiguousarray(inputs["Wk"][:, gslice], np.float32),
                "wv": np.ascontiguousarray(inputs["Wv"][:, gslice], np.float32),
                "wo": np.ascontiguousarray(inputs["Wo"][gslice, :], np.float32),
                "bq": np.ascontiguousarray(inputs["bq"][gslice], np.float32),
                "bk": np.ascontiguousarray(inputs["bk"][gslice], np.float32),
            }
        )
    nc = _get_nc(L_)
    try:
        res = run_bass_kernel_spmd(nc, in_maps, list(range(N_CORES)), trace=trace)
    except Exception:
        # transient device faults (NRT_EXEC_UNIT_UNRECOVERABLE etc.): one retry
        res = run_bass_kernel_spmd(nc, in_maps, list(range(N_CORES)), trace=trace)
    return res


def kernel(**inputs) -> np.ndarray:
    res = run_sharded(inputs)
    # host-side unshard: sum the two head-group partials per batch; add the
    # bias terms that commute out of the device computation exactly:
    # softmax rows sum to 1, so  A @ (xWv + bv) Wo + bo = A(xWv)Wo + bv@Wo + bo
    bias = (
        np.asarray(inputs["bv"], np.float32) @ np.asarray(inputs["Wo"], np.float32)
        + np.asarray(inputs["bo"], np.float32)
    )
    out = np.empty((B, L, D), dtype=np.float32)
    for b in range(B):
        out[b] = res.results[2 * b]["y"] + res.results[2 * b + 1]["y"] + bias
    return out


# revision 55
# speedup vs baseline: 1.0252x; 1.0195x over previous
"""Causal self-attention for B=4, L=2048, D=768, H=6 on 8 TRN2 NeuronCores.

Sharding: 8 cores = 4 batches x 2 head-groups (3 heads / 384 hidden each).
Each core computes, for its (batch, head-group):
  QT/KT = (x @ W{q,k})^T + b            [128d x L per head, fp32r]
  V     = x @ Wv                        [L x 384, fp32r]
  per head, per 512-wide q-group:
    S^T  = K_blk @ Q^T                  (PE, fp32r, causal block-skip)
    A^T  = exp(S^T / sqrt(128))         (ACT, masked on diagonal blocks)
    O^T += V_blk^T @ A^T                (PE)  + row-sums via ones-matmul
    O^T *= 1/sums  (sums broadcast over partitions via rank-1 matmul)
  Y_part = O @ Wo_slice                 [L x 768 partial]
Host sums the two head-group partials per batch and adds the bias terms
(bv @ Wo + bo); bq/bk are applied on-device (zero-cost per-partition add).

All matmuls run in float32r (full PE rate, ~1e-4 rel err); softmax math in
fp32. exp needs no max-subtraction: scores/sqrt(128) stay in [-10, 10] for
normally-distributed inputs, well inside fp32 exp range.
"""

import math

import numpy as np

import concourse.bacc as bacc
import concourse.mybir as mybir
import concourse.tile as tile
from concourse.bass_utils import run_bass_kernel_spmd

F32 = mybir.dt.float32
F32R = mybir.dt.float32r
EXP = mybir.ActivationFunctionType.Exp

B = 4
L = 2048
D = 768
HEADS = 6
HD = 128
HPC = 3          # heads per core
HG = HPC * HD    # 384: per-core slice of the hidden dim
CB = D // 128    # 6 contraction chunks
SCALE = 1.0 / math.sqrt(HD)
N_CORES = 8


def build_nc(L_=L):
    """Build + compile the per-core Bass program (same program on all cores)."""
    LBn = L_ // 128   # 128-row L blocks
    NQG = L_ // 512   # 512-wide q groups

    nc = bacc.Bacc("TRN2", target_bir_lowering=False, debug=False)
    x_d = nc.dram_tensor("x", [L_, D], F32, kind="ExternalInput").ap()
    wq_d = nc.dram_tensor("wq", [D, HG], F32, kind="ExternalInput").ap()
    wk_d = nc.dram_tensor("wk", [D, HG], F32, kind="ExternalInput").ap()
    wv_d = nc.dram_tensor("wv", [D, HG], F32, kind="ExternalInput").ap()
    wo_d = nc.dram_tensor("wo", [HG, D], F32, kind="ExternalInput").ap()
    bq_d = nc.dram_tensor("bq", [HG], F32, kind="ExternalInput").ap()
    bk_d = nc.dram_tensor("bk", [HG], F32, kind="ExternalInput").ap()
    ident_d = nc.dram_tensor("ident", [128, 128], F32, kind="ExternalInput").ap()
    maskf_d = nc.dram_tensor("maskf", [128, 896], F32, kind="ExternalInput").ap()
    y_d = nc.dram_tensor("y", [L_, D], F32, kind="ExternalOutput").ap()

    with tile.TileContext(nc) as tc:
        with (
            tc.tile_pool(name="persist", bufs=1) as pp,
            tc.tile_pool(name="qkv_sb", bufs=1) as pqkv,
        ):
            # constants go on the SWDGE (gpsimd) queue so the HWDGE queue's
            # first descriptors are the x chunks the PE transposes wait on
            ident = pp.tile([128, 128], F32R)
            nc.gpsimd.dma_start(ident, ident_d.bitcast(F32R))
            maskf = pp.tile([128, 896], F32R)
            bq_sb = pp.tile([128, HPC], F32)
            bk_sb = pp.tile([128, HPC], F32)
            nc.gpsimd.dma_start(bq_sb, bq_d.rearrange("(h p) -> p h", p=128))
            nc.gpsimd.dma_start(bk_sb, bk_d.rearrange("(h p) -> p h", p=128))
            # dummy exp: pulls the ACT Exp-table load off the QKV->attention
            # transition and into the startup DMA shadow
            warm = pp.tile([1, 1], F32)
            nc.scalar.activation(warm, ident[:1, :1], EXP)

            q_t = pqkv.tile([128, HPC, L_], F32R)   # Q^T: [d, (head, L)]
            k_t = pqkv.tile([128, HPC, L_], F32R)   # K^T
            v_t = pqkv.tile([128, LBn, HG], F32R)   # V:  [k-in-block, (block, hd)]
            o_t = pqkv.tile([128, HPC, L_], F32R)   # O^T (normalized)

            # ---- phase 1: load x, transpose to x^T, QKV projections ----
            with (
                tc.tile_pool(name="w_sb", bufs=1) as pw,
                tc.tile_pool(name="x_nat", bufs=8) as px,
                tc.tile_pool(name="xT", bufs=2) as pxt,
                tc.tile_pool(name="ps_t", bufs=2, space="PSUM") as ps_t,
                tc.tile_pool(name="ps_qk", bufs=2, space="PSUM") as ps_qk,
                tc.tile_pool(name="ps_v", bufs=2, space="PSUM") as ps_v,
            ):
                # per-128-row x tiles: fine-grained DMA→transpose pipelining
                def load_xb(g, b):
                    # alternate between the two HWDGE queues (SP / Activation)
                    # to parallelize descriptor generation and transfers
                    xb = px.tile([128, D], F32R, tag="xn")
                    r0 = g * 512 + b * 128
                    eng = nc.scalar if (g == 0 and b % 2 == 1) else nc.sync
                    eng.dma_start(
                        xb,
                        x_d.bitcast(F32R)[r0 : r0 + 128, :].rearrange(
                            "(o p) c -> p o c", p=128
                        )[:, 0],
                    )
                    return xb

                xbs = [load_xb(0, b) for b in range(4)]
                wq_sb = pw.tile([128, CB, HG], F32R)
                wk_sb = pw.tile([128, CB, HG], F32R)
                wv_sb = pw.tile([128, CB, HG], F32R)
                for w_sb, w_d in ((wq_sb, wq_d), (wk_sb, wk_d), (wv_sb, wv_d)):
                    nc.sync.dma_start(
                        w_sb, w_d.bitcast(F32R).rearrange("(c p) d -> p c d", p=128)
                    )

                def emit_transposes(g, xn):
                    # b-major groups: each PSUM group + copy depends on a
                    # single x row-block DMA, so the PE transposes stream in
                    # lockstep with the arriving sub-DMAs
                    xt = pxt.tile([128, CB, 512], F32R, name="xt")
                    for b in range(4):
                        for c0, cw in ((0, 4), (4, 2)):
                            pt = ps_t.tile([128, cw, 128], F32R, name="pt")
                            for ci in range(cw):
                                c = c0 + ci
                                nc.tensor.transpose(
                                    pt[:, ci, :],
                                    xn[b][:, c * 128 : (c + 1) * 128],
                                    ident,
                                )
                            nc.vector.tensor_copy(
                                xt[:, c0 : c0 + cw, b * 128 : (b + 1) * 128], pt
                            )
                    return xt

                xt = emit_transposes(0, xbs)
                for g in range(NQG):
                    if g + 1 < NQG:  # prefetch next chunk
                        xbs = [load_xb(g + 1, b) for b in range(4)]
                    qsl = slice(g * 512, (g + 1) * 512)
                    for h in range(HPC):
                        hsl = slice(h * 128, (h + 1) * 128)
                        pq = ps_qk.tile([128, 512], F32, tag="pq")
                        for c in range(CB):
                            nc.tensor.matmul(
                                pq, wq_sb[:, c, hsl], xt[:, c, :],
                                start=(c == 0), stop=(c == CB - 1),
                            )
                        nc.scalar.activation(
                            q_t[:, h, qsl], pq,
                            mybir.ActivationFunctionType.Identity,
                            bias=bq_sb[:, h : h + 1],
                        )
                        pk = ps_qk.tile([128, 512], F32, tag="pk")
                        for c in range(CB):
                            nc.tensor.matmul(
                                pk, wk_sb[:, c, hsl], xt[:, c, :],
                                start=(c == 0), stop=(c == CB - 1),
                            )
                        nc.scalar.activation(
                            k_t[:, h, qsl], pk,
                            mybir.ActivationFunctionType.Identity,
                            bias=bk_sb[:, h : h + 1],
                        )
                    # transposes for the next chunk run on the PE here, so
                    # their PSUM->SBUF copies land while the V matmuls run
                    xt_next = emit_transposes(g + 1, xbs) if g + 1 < NQG else None
                    for b in range(4):
                        lb = g * 4 + b
                        pv = ps_v.tile([128, HG], F32)
                        for c in range(CB):
                            nc.tensor.matmul(
                                pv, xt[:, c, b * 128 : (b + 1) * 128], wv_sb[:, c, :],
                                start=(c == 0), stop=(c == CB - 1),
                            )
                        nc.vector.tensor_copy(v_t[:, lb, :], pv)
                    xt = xt_next

            # ---- phase 2: attention + output projection ----
            with (
                tc.tile_pool(name="attn_sb", bufs=1) as pa,
                tc.tile_pool(name="at_pool", bufs=8) as pat,
                tc.tile_pool(name="nrm_sb", bufs=3) as pn,
                tc.tile_pool(name="y_pool", bufs=3) as py_,
                tc.tile_pool(name="ps_s", bufs=2, space="PSUM") as ps_s,
                tc.tile_pool(name="ps_o", bufs=2, space="PSUM") as ps_o,
                tc.tile_pool(name="ps_n", bufs=1, space="PSUM") as ps_n,
                tc.tile_pool(name="ps_y", bufs=1, space="PSUM") as ps_y,
            ):
                # maskf[p, c] = 1.0 if c >= p + 384 else 0.0; diagonal-block
                # mask for block i (0..3) is maskf[:, 384-128i : 896-128i].
                # maskf[:, 768:896] is all-ones: also used as the stationary
                # of the broadcast row-sum matmuls. Loaded here, off the
                # startup critical path.
                nc.sync.dma_start(maskf, maskf_d.bitcast(F32R))
                wo_sb = pa.tile([128, HPC, D], F32R)
                nc.sync.dma_start(
                    wo_sb, wo_d.bitcast(F32R).rearrange("(h p) e -> p h e", p=128)
                )
                # Flat software-pipelined stream over all (g, h, j) batches.
                # Per batch: S-matmuls -> exp (ACT) -> mask (DVE, diag only)
                # -> PV + row-sum matmuls. The S-matmuls of batch m+1 are
                # emitted before the PV of batch m, so the PE queue always
                # has an exp-independent batch in front of it, and the
                # finalize / projection work (which trails DVE results) is
                # emitted a batch or two late to avoid head-of-line blocks.
                flat = []
                for g in range(NQG):
                    nb = 2 * (g + 1)
                    order = list(range(nb))
                    for h in range(HPC):
                        for pos, j in enumerate(order):
                            flat.append((g, h, j, pos == nb - 1, pos == 0))
                state = {}
                pending = []  # (delay, closure)

                def emit_S(m):
                    g, h, j, last, first = flat[m]
                    ps = ps_s.tile([128, 2, 512], F32, tag="ps")
                    for t in range(2):
                        kb = 2 * j + t
                        i = kb - 4 * g
                        # diag block i: q-cols < 128i are fully masked -- skip
                        # them everywhere (S, exp, mask, PV, sums)
                        c0 = 128 * i if i > 0 else 0
                        nc.tensor.matmul(
                            ps[:, t, c0:],
                            k_t[:, h, kb * 128 : (kb + 1) * 128],
                            q_t[:, h, g * 512 + c0 : (g + 1) * 512],
                            start=True, stop=True,
                        )
                    state[m] = ps

                def emit_rest(m):
                    g, h, j, last, first = flat[m]
                    ps = state.pop(m)
                    if first:
                        state[("po", g, h)] = ps_o.tile([128, 512], F32, tag="po", name="po")
                        state[("sm", g, h)] = ps_n.tile([128, 512], F32, tag="nrm", name="psums")
                    po = state[("po", g, h)]
                    psums = state[("sm", g, h)]
                    at = pat.tile([128, 2, 512], F32R)
                    diag = j >= 2 * g
                    if diag:
                        # per-t exp over just the live columns
                        for t in range(2):
                            c0 = 128 * (2 * j + t - 4 * g)
                            nc.scalar.activation(
                                at[:, t, c0:], ps[:, t, c0:], EXP, scale=SCALE
                            )
                    elif last:
                        # split: halves the exp latency gating this group's
                        # finalize chain
                        nc.scalar.activation(at[:, 0, :], ps[:, 0, :], EXP, scale=SCALE)
                        nc.scalar.activation(at[:, 1, :], ps[:, 1, :], EXP, scale=SCALE)
                    else:
                        nc.scalar.activation(at, ps, EXP, scale=SCALE)
                    for t in range(2):
                        kb = 2 * j + t
                        i = kb - 4 * g
                        if i >= 0:
                            # triangle mask on the diagonal 128-block; the
                            # dead cols below it are never computed or read
                            nc.vector.tensor_mul(
                                at[:, t, 128 * i : 128 * i + 128],
                                at[:, t, 128 * i : 128 * i + 128],
                                maskf[:, 384:512],
                            )
                        st, sp = first and t == 0, last and t == 1
                        c0 = 128 * i if i > 0 else 0
                        nc.tensor.matmul(
                            po[:, c0:],
                            v_t[:, kb, h * 128 : (h + 1) * 128],
                            at[:, t, c0:],
                            start=st, stop=sp,
                        )
                        nc.tensor.matmul(
                            psums[:, c0:],
                            maskf[:, 768:896],
                            at[:, t, c0:],
                            start=st, stop=sp,
                        )

                def emit_finalize(g, h):
                    def run():
                        po = state.pop(("po", g, h))
                        psums = state.pop(("sm", g, h))
                        recip = pn.tile([128, 512], F32, tag="recip")
                        nc.vector.reciprocal(recip, psums)
                        nc.vector.tensor_mul(
                            o_t[:, h, g * 512 : (g + 1) * 512], po, recip
                        )
                    return run

                def emit_proj(g):
                    def run():
                        # the last group's projection runs exposed after all
                        # attention work; borrow the then-idle ps_s slots to
                        # triple-buffer it
                        final = g == NQG - 1
                        for b in range(4):
                            lb = g * 4 + b
                            lsl = slice(lb * 128, (lb + 1) * 128)
                            ysb = py_.tile([128, D], F32, tag="ysb")
                            for eh in range(2):
                                pool = ps_s if final and (b + eh) % 2 else ps_y
                                pyp = pool.tile(
                                    [128, 384], F32,
                                    tag="ps" if pool is ps_s else "pyp",
                                    name="pyp",
                                )
                                for h in range(HPC):
                                    nc.tensor.matmul(
                                        pyp,
                                        o_t[:, h, lsl],
                                        wo_sb[:, h, eh * 384 : (eh + 1) * 384],
                                        start=(h == 0), stop=(h == HPC - 1),
                                    )
                                nc.vector.tensor_copy(
                                    ysb[:, eh * 384 : (eh + 1) * 384], pyp
                                )
                            nc.sync.dma_start(y_d[lb * 128 : (lb + 1) * 128, :], ysb)
                    return run

                emit_S(0)
                for m in range(len(flat)):
                    if m + 1 < len(flat):
                        emit_S(m + 1)
                    nxt = []
                    for d, fn in pending:
                        if d <= 0:
                            fn()
                        else:
                            nxt.append((d - 1, fn))
                    pending = nxt
                    emit_rest(m)
                    g, h, j, last, first = flat[m]
                    if last:
                        pending.append((1, emit_finalize(g, h)))
                        if h == HPC - 1:
                            pending.append((2, emit_proj(g)))
                for d, fn in sorted(pending, key=lambda p: p[0]):
                    fn()

    nc.compile()
    return nc


_NC_CACHE = {}


def _get_nc(L_=L):
    if L_ not in _NC_CACHE:
        _NC_CACHE[L_] = build_nc(L_)
    return _NC_CACHE[L_]


def run_sharded(inputs, L_=L, trace=False):
    """Shard inputs over 8 cores, run, return (list of per-core y, results obj)."""
    x = np.ascontiguousarray(inputs["x_input"], dtype=np.float32)
    ident = np.eye(128, dtype=np.float32)
    maskf = (np.arange(896)[None, :] >= np.arange(128)[:, None] + 384).astype(
        np.float32
    )
    in_maps = []
    for c in range(N_CORES):
        b, gslice = c // 2, slice((c % 2) * HG, (c % 2) * HG + HG)
        in_maps.append(
            {
                "x": x[b],
                "ident": ident,
                "maskf": maskf,
                "wq": np.ascontiguousarray(inputs["Wq"][:, gslice], np.float32),
                "wk": np.ascontiguousarray(inputs["Wk"][:, gslice], np.float32),
                "wv": np.ascontiguousarray(inputs["Wv"][:, gslice], np.float32),
                "wo": np.ascontiguousarray(inputs["Wo"][gslice, :], np.float32),
                "bq": np.ascontiguousarray(inputs["bq"][gslice], np.float32),
                "bk": np.ascontiguousarray(inputs["bk"][gslice], np.float32),
            }
        )
    nc = _get_nc(L_)
    try:
        res = run_bass_kernel_spmd(nc, in_maps, list(range(N_CORES)), trace=trace)
    except Exception:
        # transient device faults (NRT_EXEC_UNIT_UNRECOVERABLE etc.): one retry
        res = run_bass_kernel_spmd(nc, in_maps, list(range(N_CORES)), trace=trace)
    return res


def kernel(**inputs) -> np.ndarray:
    res = run_sharded(inputs)
    # host-side unshard: sum the two head-group partials per batch; add the
    # bias terms that commute out of the device computation exactly:
    # softmax rows sum to 1, so  A @ (xWv + bv) Wo + bo = A(xWv)Wo + bv@Wo + bo
    bias = (
        np.asarray(inputs["bv"], np.float32) @ np.asarray(inputs["Wo"], np.float32)
        + np.asarray(inputs["bo"], np.float32)
    )
    out = np.empty((B, L, D), dtype=np.float32)
    for b in range(B):
        out[b] = res.results[2 * b]["y"] + res.results[2 * b + 1]["y"] + bias
    return out
